# revision 1
# baseline (speedup 1.0000x reference)
"""Trainium2 Bass kernel for nn_CNNRandomProjection (B=256, C=128, H=W=32).

Reference computation:
    y[b,c,k,w] = sum_h P[c,k,h] * x[b,c,h,w]
    y = lam * y ; y = leaky_relu(y, 0.2)
    out = gamma * (y - mean_c) * rsqrt(var_c + 1e-5) + beta     (stats over B,H,W)

Distribution: shard the CHANNEL axis across the 8 NeuronCores (16 channels
per core). BatchNorm statistics are per-channel, so each core owns the full
batch for its channels and no cross-core communication is needed.

Per core the 16 channels are processed as 4 groups of 4 channels. For each
group a 128x128 block-diagonal weight tile (4 diagonal 32x32 blocks, each
P[c].T) contracts 4 channels x 32 h at once:  psum[32i+k, col] =
sum_h W[32i+h, 32i+k] * x[32i+h, col], with col = (batch, w) packed along
the free dim.  ScalarE applies lam (runtime scale) + leaky-relu while moving
PSUM->SBUF; VectorE bn_stats accumulates per-partition mean/var in the same
pass; two tiny selector matmuls fold the stats across partitions and expand
the per-channel affine (a, b) back to partitions; a single fused
tensor_scalar (y*a + b) and a contiguous DMA store finish each tile.

The host packs x into the exact SBUF tile layout so every DMA is fully
contiguous (8 KB per partition per transfer).
"""

import numpy as np

import concourse.bass as bass
import concourse.bacc as bacc
import concourse.tile as tile
from concourse import mybir
from concourse.bass_utils import run_bass_kernel_spmd

# ---------------------------------------------------------------- constants
B, C, H, W = 256, 128, 32, 32
NCORES = 8
CLOC = C // NCORES          # channels per core = 16
BN_EPS = 1e-5
NEG_SLOPE = 0.2
F32 = mybir.dt.float32


class Cfg:
    """Geometry of the per-core kernel (parametrized so a mini version can
    run through the interpreter)."""

    def __init__(self, G=4, NJG=4, TS=2048):
        self.G = G                    # channel groups (4 channels each)
        self.NJG = NJG                # DMA tiles per group
        self.TS = TS                  # free-dim columns per tile
        self.NQ = TS // 512           # matmuls (512-col chunks) per tile
        self.NB = NJG * self.NQ * 16  # batches covered (16 batches per 512 cols)
        self.NFREE = NJG * TS         # free elements per partition per group
        self.NTOT = 32 * self.NFREE   # BN element count per channel (32 k-rows)


FULL = Cfg()
assert FULL.NB == B and FULL.G * 4 == CLOC


# ------------------------------------------------------------- bass program
def build_nc(cfg: Cfg, reps: int = 1, mode: str = "full"):
    G, NJG, TS, NQ = cfg.G, cfg.NJG, cfg.TS, cfg.NQ
    # Bacc (not raw Bass): its compile() runs generate_event_semaphores,
    # which legalizes to the TRN2 1-sync-wait-per-instruction constraint.
    nc = bacc.Bacc("TRN2", target_bir_lowering=False, debug=False)

    xt = nc.dram_tensor("xt", [G, NJG, 128, TS], F32, kind="ExternalInput")
    ct = nc.dram_tensor("ct", [128, const_cols(cfg)], F32, kind="ExternalInput")
    yt = nc.dram_tensor("yt", [G, NJG, 128, TS], F32, kind="ExternalOutput")

    with tile.TileContext(nc) as tc:
        _body(tc, {"yt": yt.ap()}, {"xt": xt.ap(), "ct": ct.ap()},
              cfg, reps=reps, mode=mode)
    nc.compile()
    return nc


def _const_offsets(cfg: Cfg):
    """Column offsets inside the packed constants panel [128, NCOLS]:
    W | lam | zero | sel | gb(rows 0:4) | selT(rows 0:4) | eps(rows 0:4)."""
    G = cfg.G
    o = {}
    o["w"] = 0
    o["lam"] = G * 128
    o["zero"] = o["lam"] + 1
    o["sel"] = o["zero"] + 1
    o["gb"] = o["sel"] + 4
    o["selT"] = o["gb"] + 2 * G
    o["eps"] = o["selT"] + 128
    o["end"] = o["eps"] + 1
    return o


def const_cols(cfg: Cfg):
    return _const_offsets(cfg)["end"]


def _body(tc, outs, ins, cfg: Cfg, reps: int = 1, mode: str = "full"):
    """Kernel body over DRAM APs (shared by the HW path and the interp test).
    reps > 1 wraps the whole body in a hardware For_i loop — used only by the
    timing bench to amplify device time above the dispatch-noise floor.
    mode: "full" = real kernel; "dmaonly" = just the load + store streams
    (garbage output) to measure the DMA roofline of this access pattern."""
    nc = tc.nc
    G, NJG, TS, NQ = cfg.G, cfg.NJG, cfg.TS, cfg.NQ
    xt, ct = ins["xt"], ins["ct"]
    yt = outs["yt"]
    off = _const_offsets(cfg)

    from contextlib import ExitStack
    with ExitStack() as ctx:
        singles = ctx.enter_context(tc.tile_pool(name="singles", bufs=1))
        xpool = ctx.enter_context(tc.tile_pool(name="xp", bufs=3))
        ypool = ctx.enter_context(tc.tile_pool(name="yp", bufs=1))
        pspool = ctx.enter_context(tc.tile_pool(name="ps", bufs=4, space="PSUM"))
        ps2 = ctx.enter_context(tc.tile_pool(name="ps2", bufs=1, space="PSUM"))
        # Scratch PSUM bank for "wait absorber" matmuls: walrus allows only a
        # single sync-wait on a Matmult (it lands on the LDWEIGHTS half), so
        # before each tile's real matmuls a dummy 1x1 matmul absorbs the
        # x-DMA semaphore wait into PE's vector clock; the real matmuls then
        # only ever carry the one PSUM-WAR wait.
        absp = ctx.enter_context(tc.tile_pool(name="absp", bufs=1, space="PSUM"))
        abs_ps = absp.tile([1, 1], F32, tag="abs", name="abs_ps")

        if reps > 1:
            ctx.enter_context(tc.For_i(0, reps, 1))

        if mode == "dmaonly":
            src = singles.tile([128, TS], F32, tag="dsrc", name="dsrc")
            nc.vector.memset(src[:, 0:1], 0.0)
            for g in range(G):
                for jg in range(NJG):
                    xtile = xpool.tile([128, TS], F32, tag="x", name=f"dx_{g}_{jg}")
                    nc.sync.dma_start(out=xtile, in_=xt[g, jg])
                    nc.gpsimd.dma_start(out=yt[g, jg], in_=src)
            return

        # One DMA brings every constant in: block-diag weights, lam broadcast,
        # a zero column, the two selector matrices, gamma/beta and eps.
        c_sb = singles.tile([128, off["end"]], F32)
        nc.sync.dma_start(out=c_sb, in_=ct)
        w_sb = c_sb[:, off["w"]:off["w"] + G * 128]
        lam_sb = c_sb[:, off["lam"]:off["lam"] + 1]
        zero_sb = c_sb[:, off["zero"]:off["zero"] + 1]
        sel_sb = c_sb[:, off["sel"]:off["sel"] + 4]
        gb_sb = c_sb[0:4, off["gb"]:off["gb"] + 2 * G]
        selT_sb = c_sb[0:4, off["selT"]:off["selT"] + 128]
        eps_sb = c_sb[0:4, off["eps"]:off["eps"] + 1]
        # ACT warmup: observe the const-DMA semaphore once so the per-tile
        # Prelu activations only ever carry the single PE sync-wait.
        act_warm = singles.tile([128, 1], F32)
        nc.scalar.activation(out=act_warm, in_=zero_sb,
                             func=mybir.ActivationFunctionType.Identity,
                             bias=zero_sb, scale=lam_sb)

        stats = singles.tile([128, G, NJG * NQ, 6], F32)

        # Per-group pipeline: BN stats are per-channel and each group owns 4
        # channels, so group g can fold its stats, normalize and store while
        # group g+1 is still loading/projecting — out-DMA overlaps in-DMA
        # and the stats-fold latency hides under neighbouring groups' DMA.
        for g in range(G):
            ytiles = []
            for jg in range(NJG):
                xtile = xpool.tile([128, TS], F32, tag="x", name=f"x_{g}_{jg}")
                nc.sync.dma_start(out=xtile, in_=xt[g, jg])
                ytile = ypool.tile([128, TS], F32, tag=f"y_{g}_{jg}",
                                   name=f"y_{g}_{jg}")
                ytiles.append(ytile)
                nc.tensor.matmul(abs_ps, xtile[0:1, 0:1], xtile[0:1, 0:1],
                                 start=True, stop=True)
                for q in range(NQ):
                    ps = pspool.tile([128, 512], F32, tag="mm", name=f"mm_{g}_{jg}_{q}")
                    nc.tensor.matmul(ps, w_sb[:, g * 128:(g + 1) * 128],
                                     xtile[:, q * 512:(q + 1) * 512],
                                     start=True, stop=True)
                    # NOTE: Prelu, not Lrelu — the HW Lrelu table ignores the
                    # alpha operand (fixed 0.01 slope); Prelu honors it.
                    nc.scalar.activation(
                        out=ytile[:, q * 512:(q + 1) * 512], in_=ps,
                        func=mybir.ActivationFunctionType.Prelu,
                        bias=zero_sb[:, :], scale=lam_sb[:, :], alpha=NEG_SLOPE)
                    nc.vector.bn_stats(out=stats[:, g, jg * NQ + q, :],
                                       in_=ytile[:, q * 512:(q + 1) * 512])

            # fold this group's stats to the per-channel affine (a, b)
            mv = singles.tile([128, 2], F32, tag=f"mv{g}", name=f"mv_{g}")
            nc.vector.bn_aggr(out=mv, in_=stats[:, g, :, :])
            # si col0 = per-partition sum, col1 = per-partition sum-of-squares
            si = singles.tile([128, 2], F32, tag=f"si{g}", name=f"si_{g}")
            nc.vector.tensor_copy(si[:, 0:1], mv[:, 0:1])
            nc.vector.tensor_mul(si[:, 1:2], mv[:, 0:1], mv[:, 0:1])
            nc.vector.tensor_add(si[:, 1:2], si[:, 1:2], mv[:, 1:2])
            nc.vector.tensor_scalar_mul(si, si, float(cfg.NFREE))
            # fold across partitions: chan[i, 0] = S[4g+i], chan[i, 1] = SS
            cps = ps2.tile([4, 2], F32, tag="cstat", name=f"cstat_{g}")
            nc.tensor.matmul(cps, sel_sb, si, start=True, stop=True)
            chan = singles.tile([4, 2], F32, tag=f"chan{g}", name=f"chan_{g}")
            nc.vector.tensor_scalar_mul(chan, cps, 1.0 / float(cfg.NTOT))
            var1 = singles.tile([4, 1], F32, tag=f"var{g}", name=f"var_{g}")
            nc.vector.tensor_mul(var1, chan[:, 0:1], chan[:, 0:1])
            nc.vector.tensor_sub(var1, chan[:, 1:2], var1)
            nc.scalar.activation(out=var1, in_=var1,
                                 func=mybir.ActivationFunctionType.Sqrt,
                                 bias=eps_sb[:, :], scale=1.0)
            nc.vector.reciprocal(var1, var1)       # 1/sqrt(var+eps)
            ab = singles.tile([4, 2], F32, tag=f"ab{g}", name=f"ab_{g}")
            nc.vector.tensor_mul(ab[:, 0:1], gb_sb[:, g:g + 1], var1)
            nc.vector.tensor_mul(ab[:, 1:2], chan[:, 0:1], ab[:, 0:1])
            nc.vector.tensor_sub(ab[:, 1:2], gb_sb[:, G + g:G + g + 1], ab[:, 1:2])
            # expand to partitions: AB[p, 0] = a[4g + p//32], AB[p, 1] = b[..]
            abps = ps2.tile([128, 2], F32, tag="abps", name=f"abps_{g}")
            nc.tensor.matmul(abps, selT_sb, ab, start=True, stop=True)
            AB = singles.tile([128, 2], F32, tag=f"AB{g}", name=f"AB_{g}")
            nc.vector.tensor_copy(AB, abps)

            # normalize this group in place and store
            for jg in range(NJG):
                ytile = ytiles[jg]
                nc.vector.tensor_scalar(
                    out=ytile, in0=ytile,
                    scalar1=AB[:, 0:1], scalar2=AB[:, 1:2],
                    op0=mybir.AluOpType.mult, op1=mybir.AluOpType.add)
                # stores go out on GPSIMD's SWDGE queue: a store waiting on
                # this group's normalize must not head-of-line-block the next
                # group's loads on the SP HWDGE queue.
                nc.gpsimd.dma_start(out=yt[g, jg], in_=ytile)


# ------------------------------------------------------------ host packing
def _pack_x_shard(xs, cfg: Cfg):
    """xs [NB, 4G, 32, 32] -> [G, NJG, 128, TS] tile layout.
    partition = 32*i + h ; col = jj*512 + bl*32 + w ; b = jg*(NQ*16) + jj*16 + bl."""
    G, NJG, NQ, TS = cfg.G, cfg.NJG, cfg.NQ, cfg.TS
    t = xs.reshape(NJG, NQ, 16, G, 4, H, W)          # [jg, jj, bl, g, i, h, w]
    t = t.transpose(3, 0, 4, 5, 1, 2, 6)             # [g, jg, i, h, jj, bl, w]
    return np.ascontiguousarray(t).reshape(G, NJG, 128, TS)


def _unpack_y_shard(ytv, cfg: Cfg):
    """[G, NJG, 128, TS] -> [NB, 4G, 32, 32]."""
    G, NJG, NQ, TS = cfg.G, cfg.NJG, cfg.NQ, cfg.TS
    t = ytv.reshape(G, NJG, 4, 32, NQ, 16, W)        # [g, jg, i, k, jj, bl, w]
    t = t.transpose(1, 4, 5, 0, 2, 3, 6)             # [jg, jj, bl, g, i, k, w]
    return t.reshape(cfg.NB, 4 * G, H, W)


def _pack_const(Pshard, lam, gamma_s, beta_s, cfg: Cfg):
    """Pack every constant the kernel needs into one [128, NCOLS] panel."""
    G = cfg.G
    off = _const_offsets(cfg)
    c = np.zeros((128, off["end"]), np.float32)
    for g in range(G):
        for i in range(4):
            c[32 * i:32 * (i + 1),
              off["w"] + g * 128 + 32 * i:off["w"] + g * 128 + 32 * (i + 1)] = \
                Pshard[4 * g + i].T
    c[:, off["lam"]] = np.float32(lam[0])
    # off["zero"] column stays 0
    sel = np.zeros((128, 4), np.float32)
    sel[np.arange(128), np.arange(128) // 32] = 1.0
    c[:, off["sel"]:off["sel"] + 4] = sel
    c[0:4, off["gb"]:off["gb"] + G] = gamma_s.reshape(G, 4).T
    c[0:4, off["gb"] + G:off["gb"] + 2 * G] = beta_s.reshape(G, 4).T
    c[0:4, off["selT"]:off["selT"] + 128] = sel.T
    c[0:4, off["eps"]] = BN_EPS
    return c


def make_in_maps(x, P, lam, gamma, beta, cfg: Cfg = FULL, ncores: int = NCORES):
    cl = 4 * cfg.G
    maps = []
    for m in range(ncores):
        sl = slice(m * cl, (m + 1) * cl)
        maps.append({
            "xt": _pack_x_shard(np.ascontiguousarray(x[:, sl]), cfg),
            "ct": _pack_const(P[sl], lam, gamma[sl], beta[sl], cfg),
        })
    return maps


_NC_CACHE = {}


def _get_nc(cfg: Cfg = FULL):
    key = (cfg.G, cfg.NJG, cfg.TS)
    if key not in _NC_CACHE:
        _NC_CACHE[key] = build_nc(cfg)
    return _NC_CACHE[key]


def run(inputs, trace=False, tmpdir=None):
    """Run on the 8 NeuronCores; returns (out, BassKernelResults)."""
    x = np.asarray(inputs["x"], np.float32)
    P = np.asarray(inputs["P"], np.float32)
    lam = np.asarray(inputs["lam"], np.float32)
    gamma = np.asarray(inputs["gamma"], np.float32)
    beta = np.asarray(inputs["beta"], np.float32)

    nc = _get_nc(FULL)
    in_maps = make_in_maps(x, P, lam, gamma, beta, FULL)
    res = run_bass_kernel_spmd(nc, in_maps, core_ids=list(range(NCORES)),
                               trace=trace, tmpdir=tmpdir)
    out = np.empty((B, C, H, W), np.float32)
    for m in range(NCORES):
        out[:, m * CLOC:(m + 1) * CLOC] = _unpack_y_shard(
            np.asarray(res.results[m]["yt"]), FULL)
    return out, res


def kernel(**inputs):
    out, _ = run(inputs)
    return out

